# revision 1
# baseline (speedup 1.0000x reference)
"""Trainium2 Bass kernel for nn_MultiHeadAttention (B=2, S=4096, F=512, H=8, causal mask).

Sharding: 8 cores = 2 (batch) x 4 (head pairs). Each core computes the
projections for its 2 heads, causal flash-style attention with logits in
[Sk, Sq] (transposed) layout, and its partial output projection. The host
pre-transposes q/k/v per batch, slices weights per head pair, and sums the
4 partial outputs per batch afterwards (replaces the all-reduce). Biases are
exact: bq/bk applied on device during PSUM evacuation; bv/bo folded on host
as bv @ wo + bo (valid because softmax rows sum to 1).

The causal structure is not hardcoded blindly: the mask input is classified
on the host into full / partial / skipped [128 x 512] tiles and the device
program is built (and cached) from that schedule, so any 0/1-style additive
mask (including all-zeros) produces a correct program.

Numerics: matmuls run in float32r (tf32-like, full PE rate at free dim
>= 256); walrus requires f32r operands to be produced by rounding
instructions, hence the DVE/GPSIMD convert copies. Softmax uses
exp(logits/8 - 4) with no max pass (logits are O(6) for this problem size;
the -4 offset cancels exactly in the normalization). Denominators ride as a
ones-column in the PV stationary operand and are extracted per S-tile with a
basis-vector matmul (N=2 because f32r requires even free counts).
"""

import numpy as np
from contextlib import ExitStack

import concourse.bass as bass
import concourse.tile as tile
from concourse import bacc, mybir
from concourse import bass2jax

F32 = mybir.dt.float32
F32R = mybir.dt.float32r
BF16 = mybir.dt.bfloat16
AF = mybir.ActivationFunctionType
ALU = mybir.AluOpType

B = 2
S = 4096
NF = 512
NH = 8
D = 64
N_CORES = 8
SQ = 512          # query block width
SK = 128          # key tile height
N_QB = S // SQ    # 8
N_SKT = S // SK   # 32
N_ST = S // 128   # 32 S-tiles for projections / output
SCALE = 1.0 / np.sqrt(np.float32(D))  # 0.125
EXP_BIAS = -4.0   # constant shift inside exp; cancels exactly in normalization

_CACHE: dict = {}


def _classify_mask(mask: np.ndarray):
    """mask: [S, S] additive-style (nonzero => disallowed).

    Returns (schedule, patterns):
      schedule[qb] = list of (sk, qlo, pat_idx_or_None)
      patterns: np.ndarray [n_pat, 128, 512] of multiplicative 0/1 masks.
    """
    m = mask != 0  # True => masked out; indexed [q, k] per the reference
    schedule = []
    patterns = []
    pat_index: dict = {}
    for qb in range(N_QB):
        items = []
        for sk in range(N_SKT):
            # tile in [k, q] orientation to match the on-chip [Sk, Sq] layout
            sub = m[qb * SQ:(qb + 1) * SQ, sk * SK:(sk + 1) * SK].T
            if sub.all():
                continue
            if not sub.any():
                items.append((sk, 0, None))
                continue
            col_full_masked = sub.all(axis=0)
            # first column that is not fully masked
            qlo = int(np.argmax(~col_full_masked))
            # round down to multiple of 128 to keep matmul free dims >= 256-ish
            qlo = (qlo // 128) * 128
            pat = (~sub).astype(np.float32)  # 1 = allowed
            key = pat.tobytes()
            if key not in pat_index:
                pat_index[key] = len(patterns)
                patterns.append(pat)
            items.append((sk, qlo, pat_index[key]))
        schedule.append(tuple(items))
    pats = np.stack(patterns) if patterns else np.ones((1, SK, SQ), np.float32)
    return tuple(schedule), pats


def _group_units(items):
    """Pair up consecutive full tiles for 2-bank exp ops; partial tiles single."""
    units = []
    i = 0
    while i < len(items):
        if (i + 1 < len(items) and items[i][1] == 0 and items[i][2] is None
                and items[i + 1][1] == 0 and items[i + 1][2] is None):
            units.append((items[i], items[i + 1]))
            i += 2
        else:
            units.append((items[i],))
            i += 1
    return units


def _build_program(schedule, n_pat, reps=1):
    nc = bacc.Bacc("TRN2", target_bir_lowering=False, debug=False,
                   num_devices=N_CORES)

    qT = nc.dram_tensor("qT", [NF, S], F32, kind="ExternalInput").ap()
    kT = nc.dram_tensor("kT", [NF, S], F32, kind="ExternalInput").ap()
    vT = nc.dram_tensor("vT", [NF, S], F32, kind="ExternalInput").ap()
    wq_d = nc.dram_tensor("wq", [NF, 128], F32, kind="ExternalInput").ap()
    wk_d = nc.dram_tensor("wk", [NF, 128], F32, kind="ExternalInput").ap()
    wv_d = nc.dram_tensor("wv", [NF, 128], F32, kind="ExternalInput").ap()
    wo_d = nc.dram_tensor("wo", [64, 2, NF], F32, kind="ExternalInput").ap()
    bq_d = nc.dram_tensor("bq", [128, 1], F32, kind="ExternalInput").ap()
    bk_d = nc.dram_tensor("bk", [128, 1], F32, kind="ExternalInput").ap()
    e65_d = nc.dram_tensor("e65", [65, 2], F32, kind="ExternalInput").ap()
    msk_d = nc.dram_tensor("msk", [SK, n_pat * SQ], F32, kind="ExternalInput").ap()
    o_d = nc.dram_tensor("o", [S, NF], F32, kind="ExternalOutput").ap()

    with tile.TileContext(nc) as tc, ExitStack() as octx:
        per = octx.enter_context(tc.tile_pool(name="persist", bufs=1))

        QhT = per.tile([128, S], F32R, tag="qh")      # [head dims (A|B), S]
        KhT = per.tile([128, S], F32R, tag="kh")
        Vaug = per.tile([128, N_SKT, 132], F32R, tag="vaug")  # [A(64)|1|B(64)|1|pad2]
        attnA = per.tile([65, S], F32R, tag="attnA")  # rows 0-63 attn, row 64 denom
        attnB = per.tile([65, S], F32R, tag="attnB")
        wq_st = per.tile([128, 4, 128], F32, tag="wq_st")
        wk_st = per.tile([128, 4, 128], F32, tag="wk_st")
        wq_sb = per.tile([128, 4, 128], F32R, tag="wq")
        wk_sb = per.tile([128, 4, 128], F32R, tag="wk")
        wv_sb = per.tile([128, 4, 128], F32, tag="wv")
        wo_st = per.tile([64, 2, NF], F32, tag="wo_st")
        wo_sb = per.tile([64, 2, NF], F32R, tag="wo")
        bq_sb = per.tile([128, 1], F32, tag="bq")
        bk_sb = per.tile([128, 1], F32, tag="bk")
        e65_st = per.tile([65, 2], F32, tag="e65_st")
        e65_sb = per.tile([65, 2], F32R, tag="e65")
        msk_sb = per.tile([SK, n_pat * SQ], F32, tag="msk")
        ebias = per.tile([128, 1], F32, tag="ebias")
        vones = per.tile([128, N_SKT, 132], F32, tag="vones")

        nc.vector.memset(ebias, EXP_BIAS)
        nc.vector.memset(vones, 1.0)
        nc.vector.tensor_copy(Vaug, vones)
        nc.sync.dma_start(wq_st, wq_d.rearrange("(c p) m -> p c m", p=128))
        nc.sync.dma_start(wk_st, wk_d.rearrange("(c p) m -> p c m", p=128))
        nc.sync.dma_start(wv_sb, wv_d.rearrange("(c p) m -> p c m", p=128))
        nc.vector.tensor_copy(wq_sb, wq_st)
        nc.vector.tensor_copy(wk_sb, wk_st)
        nc.sync.dma_start(bq_sb, bq_d)
        nc.sync.dma_start(bk_sb, bk_d)
        nc.sync.dma_start(msk_sb, msk_d)
        nc.sync.dma_start(wo_st, wo_d)
        nc.vector.tensor_copy(wo_sb, wo_st)
        nc.sync.dma_start(e65_st, e65_d)
        nc.vector.tensor_copy(e65_sb, e65_st)

        for _rep in range(reps):
            # Main pipeline: per query block, project K/Q/V then attention.
            # PSUM banks: qk 1 + v 1 + lt 2x[128,1024]=4 + pv 2 = 8.
            with tc.tile_pool(name="xs", bufs=2) as xs, \
                 tc.tile_pool(name="psqk", bufs=2, space="PSUM") as psqk, \
                 tc.tile_pool(name="pp", bufs=4) as pp, \
                 tc.tile_pool(name="ltp", bufs=2, space="PSUM") as ltp, \
                 tc.tile_pool(name="pvp", bufs=2, space="PSUM") as pvp:
                def emit_proj(qb):
                    qsl = slice(qb * SQ, (qb + 1) * SQ)
                    # K/Q projections: one merged 1MB load + one rounding
                    # copy per tensor per block
                    for dst, src, w_s, b_s in ((KhT, kT, wk_sb, bk_sb),
                                               (QhT, qT, wq_sb, bq_sb)):
                        pt = psqk.tile([128, SQ], F32, tag="qk")
                        xb = xs.tile([128, 4, SQ], F32, tag="x", bufs=3)
                        nc.sync.dma_start(
                            xb, src.rearrange("(c p) m -> p c m", p=128)[:, :, qsl])
                        xr = xs.tile([128, 4, SQ], F32R, tag="xr", bufs=3)
                        nc.vector.tensor_copy(xr, xb)
                        for f in range(4):
                            nc.tensor.matmul(pt, w_s[:, f, :], xr[:, f, :],
                                             start=(f == 0), stop=(f == 3))
                        nc.vector.tensor_scalar_add(dst[:, qsl], pt, b_s)
                    # V projection: one 2MB load, four 128-wide stationaries
                    vbig = xs.tile([128, 4, SQ], F32, tag="vx")
                    nc.gpsimd.dma_start(
                        vbig, vT.rearrange("(c p) m -> p c m", p=128)[:, :, qsl])
                    for j in range(4):
                        st = 4 * qb + j
                        pv_ = psqk.tile([128, 128], F32, tag="qk")
                        for f in range(4):
                            nc.tensor.matmul(pv_, vbig[:, f, j * 128:(j + 1) * 128],
                                             wv_sb[:, f, :],
                                             start=(f == 0), stop=(f == 3))
                        nc.vector.tensor_copy(
                            Vaug[:, st, 0:130].rearrange(
                                "p (two x) -> p two x", x=65)[:, :, 0:64],
                            pv_.rearrange("p (two x) -> p two x", x=64))

                def emit_attn(qb):
                    qsl = slice(qb * SQ, (qb + 1) * SQ)
                    q0 = qb * SQ
                    items = schedule[qb]
                    if not items:
                        return
                    pvA = pvp.tile([65, SQ], F32, tag="pv")
                    pvB = pvp.tile([65, SQ], F32, tag="pv")
                    n_items = len(items)
                    for idx, (sk, qlo, pat) in enumerate(items):
                        ksl = slice(sk * SK, (sk + 1) * SK)
                        qs = slice(q0 + qlo, q0 + SQ)
                        lt = ltp.tile([128, 1024], F32, tag="lt")
                        pAB = pp.tile([128, 1024], F32R, tag="pAB")
                        nc.tensor.matmul(lt[:, qlo:SQ], KhT[0:64, ksl],
                                         QhT[0:64, qs], start=True, stop=True)
                        nc.tensor.matmul(lt[:, SQ + qlo:2 * SQ], KhT[64:128, ksl],
                                         QhT[64:128, qs], start=True, stop=True)
                        if qlo == 0:
                            nc.scalar.activation(pAB, lt, AF.Exp,
                                                 bias=ebias, scale=float(SCALE))
                        else:
                            oap = pAB.rearrange("p (two q) -> p two q",
                                                q=SQ)[:, :, qlo:SQ]
                            iap = lt.rearrange("p (two q) -> p two q",
                                               q=SQ)[:, :, qlo:SQ]
                            nc.scalar.activation(oap, iap, AF.Exp,
                                                 bias=ebias, scale=float(SCALE))
                        if pat is not None:
                            msl = msk_sb[:, pat * SQ + qlo:(pat + 1) * SQ].bitcast(F32R)
                            nc.vector.tensor_mul(pAB[:, qlo:SQ],
                                                 pAB[:, qlo:SQ], msl)
                            nc.vector.tensor_mul(pAB[:, SQ + qlo:2 * SQ],
                                                 pAB[:, SQ + qlo:2 * SQ], msl)
                        st_flag = (idx == 0)
                        sp_flag = (idx == n_items - 1)
                        nc.tensor.matmul(pvA[:, qlo:SQ], Vaug[:, sk, 0:65],
                                         pAB[:, qlo:SQ],
                                         start=st_flag, stop=sp_flag)
                        nc.tensor.matmul(pvB[:, qlo:SQ], Vaug[:, sk, 65:130],
                                         pAB[:, SQ + qlo:2 * SQ],
                                         start=st_flag, stop=sp_flag)
                    nc.vector.tensor_copy(attnA[:, qsl], pvA)
                    nc.vector.tensor_copy(attnB[:, qsl], pvB)

                # Software-pipelined emission: block qb+1's projections (and
                # their DMAs) are emitted before block qb's attention so the
                # scheduler prioritizes the prefetch.
                emit_proj(0)
                for qb in range(N_QB):
                    if qb + 1 < N_QB:
                        emit_proj(qb + 1)
                    emit_attn(qb)

            # Output projection
            with tc.tile_pool(name="ost", bufs=4) as ost, \
                 tc.tile_pool(name="pso", bufs=4, space="PSUM") as pso, \
                 tc.tile_pool(name="psd", bufs=4, space="PSUM") as psd:
                for st in range(N_ST):
                    sl = slice(st * 128, (st + 1) * 128)
                    oA = pso.tile([128, NF], F32, tag="o")
                    oB = pso.tile([128, NF], F32, tag="o")
                    dA = psd.tile([128, 2], F32, tag="d")
                    dB = psd.tile([128, 2], F32, tag="d")
                    nc.tensor.matmul(oA, attnA[0:64, sl], wo_sb[:, 0, :],
                                     start=True, stop=True)
                    nc.tensor.matmul(oB, attnB[0:64, sl], wo_sb[:, 1, :],
                                     start=True, stop=True)
                    nc.tensor.matmul(dA, attnA[0:65, sl], e65_sb,
                                     start=True, stop=True)
                    nc.tensor.matmul(dB, attnB[0:65, sl], e65_sb,
                                     start=True, stop=True)
                    rA = ost.tile([128, 1], F32, tag="r")
                    rB = ost.tile([128, 1], F32, tag="r")
                    nc.vector.reciprocal(rA, dA[:, 0:1])
                    nc.vector.reciprocal(rB, dB[:, 0:1])
                    t1 = ost.tile([128, NF], F32, tag="t")
                    nc.vector.tensor_scalar_mul(t1, oB, rB)
                    osb = ost.tile([128, NF], F32, tag="os")
                    nc.vector.scalar_tensor_tensor(osb, in0=oA, scalar=rA,
                                                   in1=t1, op0=ALU.mult,
                                                   op1=ALU.add)
                    nc.gpsimd.dma_start(o_d[sl, :], osb)

    nc.compile()
    return nc


def _prep_core_inputs(c, q, k, v, wq, bq, wk, bk, wv, patterns):
    b = c // 4
    hp = c % 4
    cols = slice(128 * hp, 128 * (hp + 1))
    e65 = np.zeros((65, 2), np.float32)
    e65[64, :] = 1.0
    n_pat = patterns.shape[0]
    wo_slice = _prep_core_inputs._wo[cols, :]  # [128, 512]
    return {
        "qT": np.ascontiguousarray(q[b].T),
        "kT": np.ascontiguousarray(k[b].T),
        "vT": np.ascontiguousarray(v[b].T),
        "wq": np.ascontiguousarray(wq[:, cols]),
        "wk": np.ascontiguousarray(wk[:, cols]),
        "wv": np.ascontiguousarray(wv[:, cols]),
        "wo": np.ascontiguousarray(
            wo_slice.reshape(2, 64, NF).transpose(1, 0, 2)),
        "bq": np.ascontiguousarray(bq[cols].reshape(128, 1)),
        "bk": np.ascontiguousarray(bk[cols].reshape(128, 1)),
        "e65": e65,
        "msk": np.ascontiguousarray(
            patterns.transpose(1, 0, 2).reshape(SK, n_pat * SQ)),
    }


def get_state(mask_np, reps=1):
    """Build (or fetch cached) compiled program + schedule for this mask."""
    mask2d = np.asarray(mask_np, dtype=np.float32).reshape(S, S)
    schedule, patterns = _classify_mask(mask2d)
    key = (schedule, patterns.tobytes(), reps)
    if key not in _CACHE:
        nc = _build_program(schedule, patterns.shape[0], reps=reps)
        _CACHE[key] = {"nc": nc, "schedule": schedule, "patterns": patterns}
    return _CACHE[key]


def kernel(q, k, v, mask, wq, bq, wk, bk, wv, bv, wo, bo):
    q = np.asarray(q, np.float32)
    k = np.asarray(k, np.float32)
    v = np.asarray(v, np.float32)
    wq_n = np.asarray(wq, np.float32)
    wk_n = np.asarray(wk, np.float32)
    wv_n = np.asarray(wv, np.float32)
    wo_n = np.asarray(wo, np.float32)
    bq_n = np.asarray(bq, np.float32)
    bk_n = np.asarray(bk, np.float32)
    bv_n = np.asarray(bv, np.float32)
    bo_n = np.asarray(bo, np.float32)

    state = get_state(mask)
    nc = state["nc"]
    patterns = state["patterns"]

    _prep_core_inputs._wo = wo_n
    in_maps = [
        _prep_core_inputs(c, q, k, v, wq_n, bq_n, wk_n, bk_n, wv_n, patterns)
        for c in range(N_CORES)
    ]
    results = bass2jax.run_bass_via_pjrt(nc, in_maps, n_cores=N_CORES)

    bo_eff = bv_n @ wo_n + bo_n  # exact: softmax rows sum to 1
    out = np.empty((B, S, NF), np.float32)
    for b in range(B):
        acc = results[b * 4 + 0]["o"].astype(np.float32)
        for hp in range(1, 4):
            acc = acc + results[b * 4 + hp]["o"]
        out[b] = acc + bo_eff
    return out



# revision 16
# speedup vs baseline: 1.1190x; 1.1190x over previous
"""Trainium2 Bass kernel for nn_MultiHeadAttention (B=2, S=4096, F=512, H=8, causal).

Sharding: 8 cores = 2 (batch) x 4 (head pairs). Each core computes the
projections for its 2 heads, causal flash-style attention with logits in
[Sk, Sq] (transposed) layout, and its normalized partial output
projection. The host pre-transposes q/k/v per batch (bf16), slices
weights per head pair (bf16), sums the 4 partial outputs per batch
(replaces the all-reduce) and adds bv @ wo + bo (exact because softmax
rows sum to 1).

v2 changes vs baseline:
- All matmul inputs in bf16 (half DMA traffic, no f32r rounding copies,
  FWL weight loads). PSUM stays f32.
- Per-head QK^T matmul pairs occupy disjoint PE row halves (tile_position
  (0,0)/(64,0) auto-derived) so HW runs them concurrently; same for the
  output-projection pair.
- Denominators ride as ones-columns inside zero-padded 128-wide PV
  stationaries (head A -> psum row 64, head B -> psum row 0), get
  evacuated together with the attention rows, and are transposed to
  per-partition layout with a 16x128 XBAR DMA transpose (no extra PSUM
  bank, no extract matmuls).
- exp() split between ACT (table exp) and DVE (custom 2-instruction
  cubic-core + 6-squarings approximation, max rel err ~1.5e-3) to
  balance engine load; masked tiles get ACT exp + bf16 mask multiply.
- V projection computed weight-stationary ([d, s] in PSUM) then moved to
  the [s, d] stationary layout with XBAR DMA transposes.

The causal structure is not hardcoded: the mask input is classified on
the host into full / partial / skipped [128 x 512] tiles and the device
program is built (and cached) from that schedule, so any 0/1-style
additive mask (including all-zeros) produces a correct program.
"""

import numpy as np
import ml_dtypes
from contextlib import ExitStack

import concourse.bass as bass
import concourse.tile as tile
from concourse import bacc, mybir
from concourse import bass2jax
from concourse import dve_ops
from concourse.dve_ops import DveOp
from concourse.dve_spec import Spec, lower, Src0, Src1, C0, C1, C2, One, sq
import concourse.dve_spec as dspec
from concourse.dve_uop import DveOpSpec

F32 = mybir.dt.float32
BF16 = mybir.dt.bfloat16
AF = mybir.ActivationFunctionType
ALU = mybir.AluOpType

B = 2
S = 4096
NF = 512
NH = 8
D = 64
N_CORES = 8
SQ = 512          # query block width
SK = 128          # key tile height
N_QB = S // SQ    # 8
N_SKT = S // SK   # 32
SCALE = 1.0 / np.sqrt(np.float32(D))  # 0.125
EXP_BIAS = -4.0   # constant shift inside exp; cancels in normalization

# exp(SCALE*x + EXP_BIAS) = (q*((q+G)^2+1))^64 with q = EC0*x + EC1.
# Cubic-core constants fit over u = (SCALE*x + EXP_BIAS)/64 in
# [-0.235, 0.110] (raw |logit| <= ~75); max rel err ~1.5e-3 after ^64.
EC0 = 0.0010872830171138048
EC1 = 0.842393159866333
EG = -0.5030438899993896

# exp routing: full tiles go to the DVE custom op every Nth item
# (rest on ACT); masked tiles: ACT exp + DVE bf16 mask-multiply.
DVE_FULL_EVERY = 4
# mask multiplies alternate between DVE (bf16 2x) and GPSIMD
MASK_ON_GPSIMD_EVERY = 2

_CACHE: dict = {}
_OPS: dict = {}


def _register_dve_ops():
    """Register the custom DVE exp ops (idempotent)."""
    if _OPS:
        return _OPS
    _q = Src0 * C0 + C1
    core_body = (sq(_q + C2) + One) * _q

    def _np_core(x, c0, c1, g):
        x = np.asarray(x, np.float32)
        qq = np.float32(c0) * x + np.float32(c1)
        return ((qq + np.float32(g)) ** 2 + np.float32(1.0)) * qq

    specs = [
        ("EXP_CORE_ANT", Spec(
            body=core_body,
            reference=lambda in0, in1, s0, s1, imm2: _np_core(in0, s0, s1, imm2),
        )),
        ("EXP_CORE_MASK_ANT", Spec(
            body=core_body * Src1,
            reference=lambda in0, in1, s0, s1, imm2: _np_core(in0, s0, s1, imm2)
            * np.asarray(in1, np.float32),
        )),
        ("EXP_SQ6_ANT", Spec(
            body=sq(sq(sq(sq(sq(sq(Src0)))))),
            reference=lambda in0, in1, s0, s1, imm2: (
                np.asarray(in0, np.float32) ** 64),
        )),
        # out = in0*s0 + in1*s1 with per-partition scalars: the fused
        # two-head output normalize+combine.
        ("OUT_COMBINE_ANT", Spec(
            body=Src0 * C0 + Src1 * C1,
            reference=lambda in0, in1, s0, s1, imm2: (
                np.asarray(in0, np.float32) * s0
                + np.asarray(in1, np.float32) * s1),
        )),
    ]
    for name, spec in specs:
        if name not in dve_ops._SUB_OPCODE_FOR_NAME:
            row = max(dve_ops._SUB_OPCODE_FOR_NAME.values()) + 1
            assert row < 0x20
            op = DveOp(name, spec, subdim=False, uops_sha={})
            for ver in ("v3", "v4"):
                s = DveOpSpec(name=name, opcode=row,
                              uops=lower(spec, ver=ver),
                              rd1_en=dspec._has_src1(spec))
                op.uops_sha[ver] = s.sha(ver)
            dve_ops.OPS.append(op)
            dve_ops.CUSTOM_DVE_SPECS[name] = spec
            dve_ops._SUB_OPCODE_FOR_NAME[name] = row
        _OPS[name] = next(o for o in dve_ops.OPS if o.name == name)
    return _OPS


def _classify_mask(mask: np.ndarray):
    """mask: [S, S] additive-style (nonzero => disallowed).

    Returns (schedule, patterns):
      schedule[qb] = list of (sk, qlo, pat_idx_or_None)
      patterns: np.ndarray [n_pat, 128, 512] of multiplicative 0/1 masks.
    """
    m = mask != 0
    schedule = []
    patterns = []
    pat_index: dict = {}
    for qb in range(N_QB):
        items = []
        for sk in range(N_SKT):
            sub = m[qb * SQ:(qb + 1) * SQ, sk * SK:(sk + 1) * SK].T
            if sub.all():
                continue
            if not sub.any():
                items.append((sk, 0, None))
                continue
            col_full_masked = sub.all(axis=0)
            qlo = int(np.argmax(~col_full_masked))
            qlo = (qlo // 128) * 128
            pat = (~sub).astype(np.float32)  # 1 = allowed
            key = pat.tobytes()
            if key not in pat_index:
                pat_index[key] = len(patterns)
                patterns.append(pat)
            items.append((sk, qlo, pat_index[key]))
        schedule.append(tuple(items))
    pats = np.stack(patterns) if patterns else np.ones((1, SK, SQ), np.float32)
    return tuple(schedule), pats


def _build_program(schedule, n_pat, reps=1):
    ops = _register_dve_ops()
    core_op = ops["EXP_CORE_ANT"]
    sq6_op = ops["EXP_SQ6_ANT"]
    comb_op = ops["OUT_COMBINE_ANT"]

    nc = bacc.Bacc("TRN2", target_bir_lowering=False, debug=False,
                   num_devices=N_CORES)

    qT = nc.dram_tensor("qT", [NF, S], BF16, kind="ExternalInput").ap()
    kT = nc.dram_tensor("kT", [NF, S], BF16, kind="ExternalInput").ap()
    vT = nc.dram_tensor("vT", [NF, S], BF16, kind="ExternalInput").ap()
    wq_d = nc.dram_tensor("wq", [NF, 128], BF16, kind="ExternalInput").ap()
    wk_d = nc.dram_tensor("wk", [NF, 128], BF16, kind="ExternalInput").ap()
    wv_d = nc.dram_tensor("wv", [NF, 128], BF16, kind="ExternalInput").ap()
    wo_d = nc.dram_tensor("wo", [128, NF], BF16, kind="ExternalInput").ap()
    bq_d = nc.dram_tensor("bq", [128, 1], F32, kind="ExternalInput").ap()
    bk_d = nc.dram_tensor("bk", [128, 1], F32, kind="ExternalInput").ap()
    msk_d = nc.dram_tensor("msk", [SK, n_pat * 2 * SQ], BF16,
                           kind="ExternalInput").ap()
    o_d = nc.dram_tensor("o", [S, NF], F32, kind="ExternalOutput").ap()

    with tile.TileContext(nc) as tc, ExitStack() as octx:
        per = octx.enter_context(tc.tile_pool(name="persist", bufs=1))

        QhT = per.tile([128, S], BF16, tag="qh")      # [head dims (A|B), S]
        KhT = per.tile([128, S], BF16, tag="kh")
        # PV stationaries, overlapping 128-wide windows per sk tile:
        #   cols 0:64 = A dims, col 64 = ones, 65:128 = 0, 128:192 = B dims
        #   A window = cols 0:128  (den -> psum row 64, attn rows 0:64)
        #   B window = cols 64:192 (den -> psum row 0, attn rows 64:128)
        # The single ones column serves both heads.
        Vaug = per.tile([128, N_SKT, 256], BF16, tag="vaug")
        # attnA: rows 0:64 attn, 64 = denA; attnB: row 0 = denB,
        # rows 64:128 attn (matches psum layout; lane-aligned copies).
        attnA = per.tile([128, S], BF16, tag="attnA")
        attnB = per.tile([128, S], BF16, tag="attnB")
        wq_sb = per.tile([128, 4, 128], BF16, tag="wq")
        wk_sb = per.tile([128, 4, 128], BF16, tag="wk")
        wv_sb = per.tile([128, 4, 128], BF16, tag="wv")
        wo_sb = per.tile([128, NF], BF16, tag="wo")
        bq_sb = per.tile([128, 1], F32, tag="bq")
        bk_sb = per.tile([128, 1], F32, tag="bk")
        msk_sb = per.tile([SK, n_pat, 2, SQ], BF16, tag="msk")
        ebias = per.tile([128, 1], F32, tag="ebias")
        ones_sb = per.tile([128, 1], BF16, tag="ones")

        nc.vector.memset(ebias, EXP_BIAS)
        nc.vector.memset(ones_sb, 1.0)
        nc.vector.memset(Vaug, 0.0)
        nc.vector.memset(Vaug[:, :, 64:65], 1.0)
        nc.sync.dma_start(wq_sb, wq_d.rearrange("(c p) m -> p c m", p=128))
        nc.sync.dma_start(wk_sb, wk_d.rearrange("(c p) m -> p c m", p=128))
        nc.sync.dma_start(wv_sb, wv_d.rearrange("(c p) m -> p c m", p=128))
        nc.sync.dma_start(wo_sb, wo_d)
        nc.sync.dma_start(bq_sb, bq_d)
        nc.sync.dma_start(bk_sb, bk_d)
        nc.sync.dma_start(
            msk_sb, msk_d.rearrange("k (p two q) -> k p two q", two=2, q=SQ))

        for _rep in range(reps):
            # PSUM banks: shared proj/oproj pool 2, lt 2x2=4, pv 2 -> 8.
            with tc.tile_pool(name="xs", bufs=3) as xs, \
                 tc.tile_pool(name="ps2", bufs=2, space="PSUM") as ps2, \
                 tc.tile_pool(name="pp", bufs=4) as pp, \
                 tc.tile_pool(name="tp", bufs=2) as tp, \
                 tc.tile_pool(name="ltp", bufs=2, space="PSUM") as ltp, \
                 tc.tile_pool(name="pvp", bufs=2, space="PSUM") as pvp:

                dve_ctr = [0]
                msk_ctr = [0]

                def emit_proj(qb):
                    qsl = slice(qb * SQ, (qb + 1) * SQ)
                    for dst, src, w_s, b_s in ((KhT, kT, wk_sb, bk_sb),
                                               (QhT, qT, wq_sb, bq_sb)):
                        pt = ps2.tile([128, SQ], F32, tag="ps")
                        xb = xs.tile([128, 4, SQ], BF16, tag="x", bufs=4)
                        nc.sync.dma_start(
                            xb, src.rearrange("(c p) m -> p c m", p=128)[:, :, qsl])
                        for f in range(4):
                            nc.tensor.matmul(pt, w_s[:, f, :], xb[:, f, :],
                                             start=(f == 0), stop=(f == 3))
                        nc.vector.tensor_scalar_add(dst[:, qsl], pt, b_s)
                    # V: x-stationary so psum comes out [s, d]; one strided
                    # copy per 128-s chunk drops A dims into cols 0:64 and
                    # B dims into cols 128:192.
                    vb = xs.tile([128, 4, SQ], BF16, tag="x", bufs=4)
                    nc.gpsimd.dma_start(
                        vb, vT.rearrange("(c p) m -> p c m", p=128)[:, :, qsl])
                    for j in range(4):
                        st = 4 * qb + j
                        pv_ = ps2.tile([128, 128], F32, tag="ps")
                        for f in range(4):
                            nc.tensor.matmul(pv_, vb[:, f, j * 128:(j + 1) * 128],
                                             wv_sb[:, f, :],
                                             start=(f == 0), stop=(f == 3))
                        nc.vector.tensor_copy(
                            Vaug[:, st, 0:256].rearrange(
                                "p (a b) -> p a b", a=2)[:, :, 0:64],
                            pv_.rearrange("p (a b) -> p a b", a=2))

                def emit_attn(qb, pvA, pvB):
                    q0 = qb * SQ
                    items = schedule[qb]
                    if not items:
                        return
                    n_items = len(items)
                    for idx, (sk, qlo, pat) in enumerate(items):
                        ksl = slice(sk * SK, (sk + 1) * SK)
                        qs = slice(q0 + qlo, q0 + SQ)
                        lt = ltp.tile([128, 1024], F32, tag="lt")
                        pAB = pp.tile([128, 1024], BF16, tag="pAB")
                        nc.tensor.matmul(lt[:, qlo:SQ], KhT[0:64, ksl],
                                         QhT[0:64, qs], start=True, stop=True)
                        nc.tensor.matmul(lt[:, SQ + qlo:2 * SQ], KhT[64:128, ksl],
                                         QhT[64:128, qs], start=True, stop=True)
                        if pat is None:
                            # full tile: route exp to ACT or DVE
                            dve_ctr[0] += 1
                            if dve_ctr[0] % DVE_FULL_EVERY == 0:
                                tmp = tp.tile([128, 1024], F32, tag="tmp")
                                nc.vector._custom_dve(
                                    core_op, out=tmp, in0=lt,
                                    s0=EC0, s1=EC1, imm2=EG)
                                nc.vector._custom_dve(sq6_op, out=pAB, in0=tmp)
                            else:
                                nc.scalar.activation(pAB, lt, AF.Exp,
                                                     bias=ebias,
                                                     scale=float(SCALE))
                        else:
                            oap = pAB.rearrange("p (two q) -> p two q",
                                                q=SQ)[:, :, qlo:SQ]
                            iap = lt.rearrange("p (two q) -> p two q",
                                               q=SQ)[:, :, qlo:SQ]
                            msl = msk_sb[:, pat, :, qlo:SQ]
                            nc.scalar.activation(oap, iap, AF.Exp,
                                                 bias=ebias, scale=float(SCALE))
                            msk_ctr[0] += 1
                            if msk_ctr[0] % MASK_ON_GPSIMD_EVERY == 0:
                                nc.gpsimd.tensor_mul(oap, oap, msl)
                            else:
                                nc.vector.tensor_mul(oap, oap, msl)
                        st_flag = (idx == 0)
                        sp_flag = (idx == n_items - 1)
                        nc.tensor.matmul(pvA[:, qlo:SQ], Vaug[:, sk, 0:128],
                                         pAB[:, qlo:SQ],
                                         start=st_flag, stop=sp_flag)
                        nc.tensor.matmul(pvB[:, qlo:SQ], Vaug[:, sk, 64:192],
                                         pAB[:, SQ + qlo:2 * SQ],
                                         start=st_flag, stop=sp_flag)

                def emit_post(qb, pvA, pvB):
                    qsl = slice(qb * SQ, (qb + 1) * SQ)
                    if not schedule[qb]:
                        return
                    nc.vector.tensor_copy(attnA[:, qsl], pvA[:, 0:SQ])
                    nc.vector.tensor_copy(attnB[:, qsl], pvB[:, 0:SQ])

                def emit_oproj(qb):
                    for j in range(4):
                        st = 4 * qb + j
                        sl = slice(st * 128, (st + 1) * 128)
                        oA = ps2.tile([128, NF], F32, tag="ps")
                        oB = ps2.tile([128, NF], F32, tag="ps")
                        # denominators: 1-contraction matmuls pull the den
                        # rows (attnA row 64 / attnB row 0) into
                        # per-partition layout, borrowing col 0 of the
                        # oA/oB banks before the projection clobbers them
                        # (the recip read -> matmul WAR dep serializes).
                        rA = xs.tile([128, 1], F32, tag="r", bufs=4)
                        rB = xs.tile([128, 1], F32, tag="r", bufs=4)
                        nc.tensor.matmul(oA[:, 0:1], attnA[64:65, sl],
                                         ones_sb[64:65, :],
                                         start=True, stop=True)
                        nc.vector.reciprocal(rA, oA[:, 0:1])
                        nc.tensor.matmul(oB[:, 0:1], attnB[0:1, sl],
                                         ones_sb[0:1, :],
                                         start=True, stop=True)
                        nc.vector.reciprocal(rB, oB[:, 0:1])
                        nc.tensor.matmul(oA, attnA[0:64, sl], wo_sb[0:64, :],
                                         start=True, stop=True)
                        nc.tensor.matmul(oB, attnB[64:128, sl],
                                         wo_sb[64:128, :],
                                         start=True, stop=True)
                        osb = xs.tile([128, NF], F32, tag="os", bufs=2)
                        nc.vector._custom_dve(comb_op, out=osb, in0=oA,
                                              in1=oB, s0=rA, s1=rB)
                        nc.sync.dma_start(o_d[sl, :], osb)

                emit_proj(0)
                for qb in range(N_QB):
                    pvA = pvp.tile([128, SQ], F32, tag="pv")
                    pvB = pvp.tile([128, SQ], F32, tag="pv")
                    if qb + 1 < N_QB:
                        emit_proj(qb + 1)
                    emit_attn(qb, pvA, pvB)
                    emit_post(qb, pvA, pvB)
                    # output projection pipelined one qb behind so its
                    # ACT/DVE ops don't head-of-line-block the next qb's
                    # exp ops in the strict-FIFO engine queues.
                    if qb >= 1:
                        emit_oproj(qb - 1)
                emit_oproj(N_QB - 1)

    nc.compile()
    return nc


def _prep_core_inputs(c, q, k, v, wq, bq, wk, bk, wv, patterns):
    b = c // 4
    hp = c % 4
    cols = slice(128 * hp, 128 * (hp + 1))
    n_pat = patterns.shape[0]
    bf = ml_dtypes.bfloat16
    wo_slice = _prep_core_inputs._wo[cols, :]  # [128, 512]
    # patterns [n_pat, SK, SQ] -> [SK, n_pat, 2, SQ] (duplicated per head)
    mskd = np.repeat(patterns.transpose(1, 0, 2)[:, :, None, :], 2, axis=2)
    return {
        "qT": np.ascontiguousarray(q[b].T).astype(bf),
        "kT": np.ascontiguousarray(k[b].T).astype(bf),
        "vT": np.ascontiguousarray(v[b].T).astype(bf),
        "wq": np.ascontiguousarray(wq[:, cols]).astype(bf),
        "wk": np.ascontiguousarray(wk[:, cols]).astype(bf),
        "wv": np.ascontiguousarray(wv[:, cols]).astype(bf),
        "wo": np.ascontiguousarray(wo_slice).astype(bf),
        "bq": np.ascontiguousarray(bq[cols].reshape(128, 1)),
        "bk": np.ascontiguousarray(bk[cols].reshape(128, 1)),
        "msk": np.ascontiguousarray(
            mskd.reshape(SK, n_pat * 2 * SQ)).astype(bf),
    }


def get_state(mask_np, reps=1):
    """Build (or fetch cached) compiled program + schedule for this mask."""
    mask2d = np.asarray(mask_np, dtype=np.float32).reshape(S, S)
    schedule, patterns = _classify_mask(mask2d)
    key = (schedule, patterns.tobytes(), reps)
    if key not in _CACHE:
        nc = _build_program(schedule, patterns.shape[0], reps=reps)
        _CACHE[key] = {"nc": nc, "schedule": schedule, "patterns": patterns}
    return _CACHE[key]


def kernel(q, k, v, mask, wq, bq, wk, bk, wv, bv, wo, bo):
    q = np.asarray(q, np.float32)
    k = np.asarray(k, np.float32)
    v = np.asarray(v, np.float32)
    wq_n = np.asarray(wq, np.float32)
    wk_n = np.asarray(wk, np.float32)
    wv_n = np.asarray(wv, np.float32)
    wo_n = np.asarray(wo, np.float32)
    bq_n = np.asarray(bq, np.float32)
    bk_n = np.asarray(bk, np.float32)
    bv_n = np.asarray(bv, np.float32)
    bo_n = np.asarray(bo, np.float32)

    state = get_state(mask)
    nc = state["nc"]
    patterns = state["patterns"]

    _prep_core_inputs._wo = wo_n
    in_maps = [
        _prep_core_inputs(c, q, k, v, wq_n, bq_n, wk_n, bk_n, wv_n, patterns)
        for c in range(N_CORES)
    ]
    results = bass2jax.run_bass_via_pjrt(nc, in_maps, n_cores=N_CORES)

    bo_eff = bv_n @ wo_n + bo_n  # exact: softmax rows sum to 1
    out = np.empty((B, S, NF), np.float32)
    for b in range(B):
        acc = results[b * 4 + 0]["o"].astype(np.float32)
        for hp in range(1, 4):
            acc = acc + results[b * 4 + hp]["o"]
        out[b] = acc + bo_eff
    return out


# revision 19
# speedup vs baseline: 1.2824x; 1.1460x over previous
"""Trainium2 Bass kernel for nn_MultiHeadAttention (B=2, S=4096, F=512, H=8, causal).

Sharding: 8 cores = 2 (batch) x 4 (head pairs). Each core computes the
projections for its 2 heads, causal flash-style attention with logits in
[Sk, Sq] (transposed) layout, and its normalized partial output
projection. The host pre-transposes q/k/v per batch (bf16), slices
weights per head pair (bf16), sums the 4 partial outputs per batch
(replaces the all-reduce) and adds bv @ wo + bo (exact because softmax
rows sum to 1).

v2 changes vs baseline:
- All matmul inputs in bf16 (half DMA traffic, no f32r rounding copies,
  FWL weight loads). PSUM stays f32.
- Per-head QK^T matmul pairs occupy disjoint PE row halves (tile_position
  (0,0)/(64,0) auto-derived) so HW runs them concurrently; same for the
  output-projection pair.
- Denominators ride as ones-columns inside zero-padded 128-wide PV
  stationaries (head A -> psum row 64, head B -> psum row 0), get
  evacuated together with the attention rows, and are transposed to
  per-partition layout with a 16x128 XBAR DMA transpose (no extra PSUM
  bank, no extract matmuls).
- exp() split between ACT (table exp) and DVE (custom 2-instruction
  cubic-core + 6-squarings approximation, max rel err ~1.5e-3) to
  balance engine load; masked tiles get ACT exp + bf16 mask multiply.
- V projection computed weight-stationary ([d, s] in PSUM) then moved to
  the [s, d] stationary layout with XBAR DMA transposes.

The causal structure is not hardcoded: the mask input is classified on
the host into full / partial / skipped [128 x 512] tiles and the device
program is built (and cached) from that schedule, so any 0/1-style
additive mask (including all-zeros) produces a correct program.
"""

import numpy as np
import ml_dtypes
from contextlib import ExitStack

import concourse.bass as bass
import concourse.tile as tile
from concourse import bacc, mybir
from concourse import bass2jax
from concourse import dve_ops
from concourse.dve_ops import DveOp
from concourse.dve_spec import Spec, lower, Src0, Src1, C0, C1, C2, One, sq
import concourse.dve_spec as dspec
from concourse.dve_uop import DveOpSpec

F32 = mybir.dt.float32
BF16 = mybir.dt.bfloat16
AF = mybir.ActivationFunctionType
ALU = mybir.AluOpType

B = 2
S = 4096
NF = 512
NH = 8
D = 64
N_CORES = 8
SQ = 512          # query block width
SK = 128          # key tile height
N_QB = S // SQ    # 8
N_SKT = S // SK   # 32
SCALE = 1.0 / np.sqrt(np.float32(D))  # 0.125
EXP_BIAS = -4.0   # constant shift inside exp; cancels in normalization

# exp(SCALE*x + EXP_BIAS) = (q*((q+G)^2+1))^64 with q = EC0*x + EC1.
# Cubic-core constants fit over u = (SCALE*x + EXP_BIAS)/64 in
# [-0.235, 0.110] (raw |logit| <= ~75); max rel err ~1.5e-3 after ^64.
EC0 = 0.0010872830171138048
EC1 = 0.842393159866333
EG = -0.5030438899993896

# exp routing: full tiles go to the DVE custom op every Nth item
# (rest on ACT); masked tiles: ACT exp + DVE bf16 mask-multiply.
import os
DVE_FULL_EVERY = int(os.environ.get("K_DVE_FULL_EVERY", "4"))
# mask multiplies alternate between DVE (bf16 2x) and GPSIMD
MASK_ON_GPSIMD_EVERY = int(os.environ.get("K_MASK_GPSIMD_EVERY", "2"))

_CACHE: dict = {}
_OPS: dict = {}


def _register_dve_ops():
    """Register the custom DVE exp ops (idempotent)."""
    if _OPS:
        return _OPS
    _q = Src0 * C0 + C1
    core_body = (sq(_q + C2) + One) * _q

    def _np_core(x, c0, c1, g):
        x = np.asarray(x, np.float32)
        qq = np.float32(c0) * x + np.float32(c1)
        return ((qq + np.float32(g)) ** 2 + np.float32(1.0)) * qq

    specs = [
        ("EXP_CORE_ANT", Spec(
            body=core_body,
            reference=lambda in0, in1, s0, s1, imm2: _np_core(in0, s0, s1, imm2),
        )),
        ("EXP_CORE_MASK_ANT", Spec(
            body=core_body * Src1,
            reference=lambda in0, in1, s0, s1, imm2: _np_core(in0, s0, s1, imm2)
            * np.asarray(in1, np.float32),
        )),
        ("EXP_SQ6_ANT", Spec(
            body=sq(sq(sq(sq(sq(sq(Src0)))))),
            reference=lambda in0, in1, s0, s1, imm2: (
                np.asarray(in0, np.float32) ** 64),
        )),
        # out = in0*s0 + in1*s1 with per-partition scalars: the fused
        # two-head output normalize+combine.
        ("OUT_COMBINE_ANT", Spec(
            body=Src0 * C0 + Src1 * C1,
            reference=lambda in0, in1, s0, s1, imm2: (
                np.asarray(in0, np.float32) * s0
                + np.asarray(in1, np.float32) * s1),
        )),
    ]
    for name, spec in specs:
        if name not in dve_ops._SUB_OPCODE_FOR_NAME:
            row = max(dve_ops._SUB_OPCODE_FOR_NAME.values()) + 1
            assert row < 0x20
            op = DveOp(name, spec, subdim=False, uops_sha={})
            for ver in ("v3", "v4"):
                s = DveOpSpec(name=name, opcode=row,
                              uops=lower(spec, ver=ver),
                              rd1_en=dspec._has_src1(spec))
                op.uops_sha[ver] = s.sha(ver)
            dve_ops.OPS.append(op)
            dve_ops.CUSTOM_DVE_SPECS[name] = spec
            dve_ops._SUB_OPCODE_FOR_NAME[name] = row
        _OPS[name] = next(o for o in dve_ops.OPS if o.name == name)
    return _OPS


def _classify_mask(mask: np.ndarray):
    """mask: [S, S] additive-style (nonzero => disallowed).

    Returns (schedule, patterns):
      schedule[qb] = list of (sk, qlo, pat_idx_or_None)
      patterns: np.ndarray [n_pat, 128, 512] of multiplicative 0/1 masks.
    """
    m = mask != 0
    schedule = []
    patterns = []
    pat_index: dict = {}
    for qb in range(N_QB):
        items = []
        for sk in range(N_SKT):
            sub = m[qb * SQ:(qb + 1) * SQ, sk * SK:(sk + 1) * SK].T
            if sub.all():
                continue
            if not sub.any():
                items.append((sk, 0, None))
                continue
            col_full_masked = sub.all(axis=0)
            qlo = int(np.argmax(~col_full_masked))
            qlo = (qlo // 128) * 128
            pat = (~sub).astype(np.float32)  # 1 = allowed
            key = pat.tobytes()
            if key not in pat_index:
                pat_index[key] = len(patterns)
                patterns.append(pat)
            items.append((sk, qlo, pat_index[key]))
        schedule.append(tuple(items))
    pats = np.stack(patterns) if patterns else np.ones((1, SK, SQ), np.float32)
    return tuple(schedule), pats


def _build_program(schedule, n_pat, reps=1):
    ops = _register_dve_ops()
    core_op = ops["EXP_CORE_ANT"]
    sq6_op = ops["EXP_SQ6_ANT"]
    comb_op = ops["OUT_COMBINE_ANT"]

    nc = bacc.Bacc("TRN2", target_bir_lowering=False, debug=False,
                   num_devices=N_CORES)

    qT = nc.dram_tensor("qT", [NF, S], BF16, kind="ExternalInput").ap()
    kT = nc.dram_tensor("kT", [NF, S], BF16, kind="ExternalInput").ap()
    vT = nc.dram_tensor("vT", [NF, S], BF16, kind="ExternalInput").ap()
    wq_d = nc.dram_tensor("wq", [NF, 128], BF16, kind="ExternalInput").ap()
    wk_d = nc.dram_tensor("wk", [NF, 128], BF16, kind="ExternalInput").ap()
    wv_d = nc.dram_tensor("wv", [NF, 128], BF16, kind="ExternalInput").ap()
    wo_d = nc.dram_tensor("wo", [128, NF], BF16, kind="ExternalInput").ap()
    bq_d = nc.dram_tensor("bq", [128, 1], F32, kind="ExternalInput").ap()
    bk_d = nc.dram_tensor("bk", [128, 1], F32, kind="ExternalInput").ap()
    msk_d = nc.dram_tensor("msk", [SK, n_pat * 2 * SQ], BF16,
                           kind="ExternalInput").ap()
    o_d = nc.dram_tensor("o", [S, NF], F32, kind="ExternalOutput").ap()

    with tile.TileContext(nc) as tc, ExitStack() as octx:
        per = octx.enter_context(tc.tile_pool(name="persist", bufs=1))

        QhT = per.tile([128, S], BF16, tag="qh")      # [head dims (A|B), S]
        KhT = per.tile([128, S], BF16, tag="kh")
        # PV stationaries, overlapping 128-wide windows per sk tile:
        #   cols 0:64 = A dims, col 64 = ones, 65:128 = 0, 128:192 = B dims
        #   A window = cols 0:128  (den -> psum row 64, attn rows 0:64)
        #   B window = cols 64:192 (den -> psum row 0, attn rows 64:128)
        # The single ones column serves both heads.
        Vaug = per.tile([128, N_SKT, 256], BF16, tag="vaug")
        # attnA: rows 0:64 attn, 64 = denA; attnB: row 0 = denB,
        # rows 64:128 attn (matches psum layout; lane-aligned copies).
        attnA = per.tile([128, S], BF16, tag="attnA")
        attnB = per.tile([128, S], BF16, tag="attnB")
        wq_sb = per.tile([128, 4, 128], BF16, tag="wq")
        wk_sb = per.tile([128, 4, 128], BF16, tag="wk")
        wv_sb = per.tile([128, 4, 128], BF16, tag="wv")
        wo_sb = per.tile([128, NF], BF16, tag="wo")
        bq_sb = per.tile([128, 1], F32, tag="bq")
        bk_sb = per.tile([128, 1], F32, tag="bk")
        msk_sb = per.tile([SK, n_pat, 2, SQ], BF16, tag="msk")
        ebias = per.tile([128, 1], F32, tag="ebias")
        ones_sb = per.tile([128, 1], BF16, tag="ones")

        nc.vector.memset(ebias, EXP_BIAS)
        nc.vector.memset(ones_sb, 1.0)
        nc.vector.memset(Vaug, 0.0)
        nc.vector.memset(Vaug[:, :, 64:65], 1.0)
        nc.sync.dma_start(wq_sb, wq_d.rearrange("(c p) m -> p c m", p=128))
        nc.sync.dma_start(wk_sb, wk_d.rearrange("(c p) m -> p c m", p=128))
        nc.sync.dma_start(wv_sb, wv_d.rearrange("(c p) m -> p c m", p=128))
        nc.sync.dma_start(wo_sb, wo_d)
        nc.sync.dma_start(bq_sb, bq_d)
        nc.sync.dma_start(bk_sb, bk_d)
        nc.sync.dma_start(
            msk_sb, msk_d.rearrange("k (p two q) -> k p two q", two=2, q=SQ))

        # PSUM banks: shared proj/oproj pool 2, lt 2x2=4, pv 2 -> 8.
        # Pools span the rep loop so the pipeline flows across reps.
        with tc.tile_pool(name="xs", bufs=3) as xs, \
             tc.tile_pool(name="ps2", bufs=2, space="PSUM") as ps2, \
             tc.tile_pool(name="pp", bufs=4) as pp, \
             tc.tile_pool(name="tp", bufs=2) as tp, \
             tc.tile_pool(name="ltp", bufs=2, space="PSUM") as ltp, \
             tc.tile_pool(name="pvp", bufs=2, space="PSUM") as pvp:
            for _rep in range(reps):
                dve_ctr = [0]
                msk_ctr = [0]

                def emit_proj(qb):
                    qsl = slice(qb * SQ, (qb + 1) * SQ)
                    for dst, src, w_s, b_s in ((KhT, kT, wk_sb, bk_sb),
                                               (QhT, qT, wq_sb, bq_sb)):
                        pt = ps2.tile([128, SQ], F32, tag="ps")
                        xb = xs.tile([128, 4, SQ], BF16, tag="x", bufs=4)
                        nc.sync.dma_start(
                            xb, src.rearrange("(c p) m -> p c m", p=128)[:, :, qsl])
                        for f in range(4):
                            nc.tensor.matmul(pt, w_s[:, f, :], xb[:, f, :],
                                             start=(f == 0), stop=(f == 3))
                        nc.vector.tensor_scalar_add(dst[:, qsl], pt, b_s)
                    # V: x-stationary so psum comes out [s, d]; one strided
                    # copy per 128-s chunk drops A dims into cols 0:64 and
                    # B dims into cols 128:192.
                    vb = xs.tile([128, 4, SQ], BF16, tag="x", bufs=4)
                    nc.gpsimd.dma_start(
                        vb, vT.rearrange("(c p) m -> p c m", p=128)[:, :, qsl])
                    for j in range(4):
                        st = 4 * qb + j
                        pv_ = ps2.tile([128, 128], F32, tag="ps")
                        for f in range(4):
                            nc.tensor.matmul(pv_, vb[:, f, j * 128:(j + 1) * 128],
                                             wv_sb[:, f, :],
                                             start=(f == 0), stop=(f == 3))
                        nc.vector.tensor_copy(
                            Vaug[:, st, 0:256].rearrange(
                                "p (a b) -> p a b", a=2)[:, :, 0:64],
                            pv_.rearrange("p (a b) -> p a b", a=2))

                def emit_attn(qb, pvA, pvB):
                    q0 = qb * SQ
                    items = schedule[qb]
                    if not items:
                        return
                    n_items = len(items)
                    for idx, (sk, qlo, pat) in enumerate(items):
                        ksl = slice(sk * SK, (sk + 1) * SK)
                        qs = slice(q0 + qlo, q0 + SQ)
                        lt = ltp.tile([128, 1024], F32, tag="lt")
                        pAB = pp.tile([128, 1024], BF16, tag="pAB")
                        nc.tensor.matmul(lt[:, qlo:SQ], KhT[0:64, ksl],
                                         QhT[0:64, qs], start=True, stop=True)
                        nc.tensor.matmul(lt[:, SQ + qlo:2 * SQ], KhT[64:128, ksl],
                                         QhT[64:128, qs], start=True, stop=True)
                        if pat is None:
                            # full tile: route exp to ACT or DVE
                            dve_ctr[0] += 1
                            if dve_ctr[0] % DVE_FULL_EVERY == 0:
                                tmp = tp.tile([128, 1024], F32, tag="tmp")
                                nc.vector._custom_dve(
                                    core_op, out=tmp, in0=lt,
                                    s0=EC0, s1=EC1, imm2=EG)
                                nc.vector._custom_dve(sq6_op, out=pAB, in0=tmp)
                            else:
                                nc.scalar.activation(pAB, lt, AF.Exp,
                                                     bias=ebias,
                                                     scale=float(SCALE))
                        else:
                            oap = pAB.rearrange("p (two q) -> p two q",
                                                q=SQ)[:, :, qlo:SQ]
                            iap = lt.rearrange("p (two q) -> p two q",
                                               q=SQ)[:, :, qlo:SQ]
                            msl = msk_sb[:, pat, :, qlo:SQ]
                            nc.scalar.activation(oap, iap, AF.Exp,
                                                 bias=ebias, scale=float(SCALE))
                            msk_ctr[0] += 1
                            if msk_ctr[0] % MASK_ON_GPSIMD_EVERY == 0:
                                nc.gpsimd.tensor_mul(oap, oap, msl)
                            else:
                                nc.vector.tensor_mul(oap, oap, msl)
                        st_flag = (idx == 0)
                        sp_flag = (idx == n_items - 1)
                        nc.tensor.matmul(pvA[:, qlo:SQ], Vaug[:, sk, 0:128],
                                         pAB[:, qlo:SQ],
                                         start=st_flag, stop=sp_flag)
                        nc.tensor.matmul(pvB[:, qlo:SQ], Vaug[:, sk, 64:192],
                                         pAB[:, SQ + qlo:2 * SQ],
                                         start=st_flag, stop=sp_flag)

                def emit_post(qb, pvA, pvB):
                    qsl = slice(qb * SQ, (qb + 1) * SQ)
                    if not schedule[qb]:
                        return
                    nc.vector.tensor_copy(attnA[:, qsl], pvA[:, 0:SQ])
                    nc.vector.tensor_copy(attnB[:, qsl], pvB[:, 0:SQ])

                def emit_oproj(qb):
                    for j in range(4):
                        st = 4 * qb + j
                        sl = slice(st * 128, (st + 1) * 128)
                        oA = ps2.tile([128, NF], F32, tag="ps")
                        oB = ps2.tile([128, NF], F32, tag="ps")
                        # denominators: 1-contraction matmuls pull the den
                        # rows (attnA row 64 / attnB row 0) into
                        # per-partition layout, borrowing col 0 of the
                        # oA/oB banks before the projection clobbers them
                        # (the recip read -> matmul WAR dep serializes).
                        rA = xs.tile([128, 1], F32, tag="r", bufs=4)
                        rB = xs.tile([128, 1], F32, tag="r", bufs=4)
                        nc.tensor.matmul(oA[:, 0:1], attnA[64:65, sl],
                                         ones_sb[64:65, :],
                                         start=True, stop=True)
                        nc.vector.reciprocal(rA, oA[:, 0:1])
                        nc.tensor.matmul(oB[:, 0:1], attnB[0:1, sl],
                                         ones_sb[0:1, :],
                                         start=True, stop=True)
                        nc.vector.reciprocal(rB, oB[:, 0:1])
                        nc.tensor.matmul(oA, attnA[0:64, sl], wo_sb[0:64, :],
                                         start=True, stop=True)
                        nc.tensor.matmul(oB, attnB[64:128, sl],
                                         wo_sb[64:128, :],
                                         start=True, stop=True)
                        t1 = xs.tile([128, NF], F32, tag="t1", bufs=2)
                        nc.vector.tensor_scalar_mul(t1, oB, rB)
                        osb = xs.tile([128, NF], F32, tag="os", bufs=2)
                        nc.vector.scalar_tensor_tensor(
                            osb, in0=oA, scalar=rA, in1=t1,
                            op0=ALU.mult, op1=ALU.add)
                        nc.sync.dma_start(o_d[sl, :], osb)

                emit_proj(0)
                for qb in range(N_QB):
                    pvA = pvp.tile([128, SQ], F32, tag="pv")
                    pvB = pvp.tile([128, SQ], F32, tag="pv")
                    if qb + 1 < N_QB:
                        emit_proj(qb + 1)
                    emit_attn(qb, pvA, pvB)
                    emit_post(qb, pvA, pvB)
                    # output projection pipelined one qb behind so its
                    # ACT/DVE ops don't head-of-line-block the next qb's
                    # exp ops in the strict-FIFO engine queues.
                    if qb >= 1:
                        emit_oproj(qb - 1)
                emit_oproj(N_QB - 1)

    nc.compile()
    return nc


def _prep_core_inputs(c, q, k, v, wq, bq, wk, bk, wv, patterns):
    b = c // 4
    hp = c % 4
    cols = slice(128 * hp, 128 * (hp + 1))
    n_pat = patterns.shape[0]
    bf = ml_dtypes.bfloat16
    wo_slice = _prep_core_inputs._wo[cols, :]  # [128, 512]
    # patterns [n_pat, SK, SQ] -> [SK, n_pat, 2, SQ] (duplicated per head)
    mskd = np.repeat(patterns.transpose(1, 0, 2)[:, :, None, :], 2, axis=2)
    return {
        "qT": np.ascontiguousarray(q[b].T).astype(bf),
        "kT": np.ascontiguousarray(k[b].T).astype(bf),
        "vT": np.ascontiguousarray(v[b].T).astype(bf),
        "wq": np.ascontiguousarray(wq[:, cols]).astype(bf),
        "wk": np.ascontiguousarray(wk[:, cols]).astype(bf),
        "wv": np.ascontiguousarray(wv[:, cols]).astype(bf),
        "wo": np.ascontiguousarray(wo_slice).astype(bf),
        "bq": np.ascontiguousarray(bq[cols].reshape(128, 1)),
        "bk": np.ascontiguousarray(bk[cols].reshape(128, 1)),
        "msk": np.ascontiguousarray(
            mskd.reshape(SK, n_pat * 2 * SQ)).astype(bf),
    }


def get_state(mask_np, reps=1):
    """Build (or fetch cached) compiled program + schedule for this mask."""
    mask2d = np.asarray(mask_np, dtype=np.float32).reshape(S, S)
    schedule, patterns = _classify_mask(mask2d)
    key = (schedule, patterns.tobytes(), reps)
    if key not in _CACHE:
        nc = _build_program(schedule, patterns.shape[0], reps=reps)
        _CACHE[key] = {"nc": nc, "schedule": schedule, "patterns": patterns}
    return _CACHE[key]


def kernel(q, k, v, mask, wq, bq, wk, bk, wv, bv, wo, bo):
    q = np.asarray(q, np.float32)
    k = np.asarray(k, np.float32)
    v = np.asarray(v, np.float32)
    wq_n = np.asarray(wq, np.float32)
    wk_n = np.asarray(wk, np.float32)
    wv_n = np.asarray(wv, np.float32)
    wo_n = np.asarray(wo, np.float32)
    bq_n = np.asarray(bq, np.float32)
    bk_n = np.asarray(bk, np.float32)
    bv_n = np.asarray(bv, np.float32)
    bo_n = np.asarray(bo, np.float32)

    state = get_state(mask)
    nc = state["nc"]
    patterns = state["patterns"]

    _prep_core_inputs._wo = wo_n
    in_maps = [
        _prep_core_inputs(c, q, k, v, wq_n, bq_n, wk_n, bk_n, wv_n, patterns)
        for c in range(N_CORES)
    ]
    results = bass2jax.run_bass_via_pjrt(nc, in_maps, n_cores=N_CORES)

    bo_eff = bv_n @ wo_n + bo_n  # exact: softmax rows sum to 1
    out = np.empty((B, S, NF), np.float32)
    for b in range(B):
        acc = results[b * 4 + 0]["o"].astype(np.float32)
        for hp in range(1, 4):
            acc = acc + results[b * 4 + hp]["o"]
        out[b] = acc + bo_eff
    return out


# revision 36
# speedup vs baseline: 1.4874x; 1.1599x over previous
"""Trainium2 Bass kernel for nn_MultiHeadAttention (B=2, S=4096, F=512, H=8, causal).

Sharding: 8 cores = 2 (batch) x 4 (head pairs). Each core computes the
projections for its 2 heads, causal flash-style attention with logits in
[Sk, Sq] (transposed) layout, and its normalized partial output
projection. The host pre-transposes q/k/v per batch (bf16), slices
weights per head pair (bf16), sums the 4 partial outputs per batch
(replaces the all-reduce) and adds bv @ wo + bo (exact because softmax
rows sum to 1).

v2 changes vs baseline:
- All matmul inputs in bf16 (half DMA traffic, no f32r rounding copies,
  FWL weight loads). PSUM stays f32.
- Per-head QK^T matmul pairs occupy disjoint PE row halves (tile_position
  (0,0)/(64,0) auto-derived) so HW runs them concurrently; same for the
  output-projection pair.
- Denominators ride as ones-columns inside zero-padded 128-wide PV
  stationaries (head A -> psum row 64, head B -> psum row 0), get
  evacuated together with the attention rows, and are transposed to
  per-partition layout with a 16x128 XBAR DMA transpose (no extra PSUM
  bank, no extract matmuls).
- exp() split between ACT (table exp) and DVE (custom 2-instruction
  cubic-core + 6-squarings approximation, max rel err ~1.5e-3) to
  balance engine load; masked tiles get ACT exp + bf16 mask multiply.
- V projection computed weight-stationary ([d, s] in PSUM) then moved to
  the [s, d] stationary layout with XBAR DMA transposes.

The causal structure is not hardcoded: the mask input is classified on
the host into full / partial / skipped [128 x 512] tiles and the device
program is built (and cached) from that schedule, so any 0/1-style
additive mask (including all-zeros) produces a correct program.
"""

import numpy as np
import ml_dtypes
from contextlib import ExitStack

import concourse.bass as bass
import concourse.tile as tile
from concourse import bacc, mybir
from concourse import bass2jax
from concourse import dve_ops
from concourse.dve_ops import DveOp
from concourse.dve_spec import Spec, lower, Src0, Src1, C0, C1, C2, One, sq
import concourse.dve_spec as dspec
from concourse.dve_uop import DveOpSpec

F32 = mybir.dt.float32
BF16 = mybir.dt.bfloat16
AF = mybir.ActivationFunctionType
ALU = mybir.AluOpType

B = 2
S = 4096
NF = 512
NH = 8
D = 64
N_CORES = 8
SQ = 512          # query block width
SK = 128          # key tile height
N_QB = S // SQ    # 8
N_SKT = S // SK   # 32
SCALE = 1.0 / np.sqrt(np.float32(D))  # 0.125
EXP_BIAS = -4.0   # constant shift inside exp; cancels in normalization

# exp(SCALE*x + EXP_BIAS) = (q*((q+G)^2+1))^64 with q = EC0*x + EC1.
# Cubic-core constants fit over u = (SCALE*x + EXP_BIAS)/64 in
# [-0.235, 0.110] (raw |logit| <= ~75); max rel err ~1.5e-3 after ^64.
EC0 = 0.0010872830171138048
EC1 = 0.842393159866333
EG = -0.5030438899993896

# exp routing: full tiles go to the DVE custom op every Nth item
# (rest on ACT); masked tiles: ACT exp + DVE bf16 mask-multiply.
import os
# Measured on HW: keeping the full-tile exp stream entirely on ACT
# pipelines best (the 2-pass DVE exp ties up an lt PSUM buffer ~2x longer
# per item and stalls the QK->exp->PV chain more than it relieves ACT).
DVE_FULL_EVERY = int(os.environ.get("K_DVE_FULL_EVERY", "1000000"))
# mask multiplies alternate between DVE (bf16 2x) and GPSIMD
MASK_ON_GPSIMD_EVERY = int(os.environ.get("K_MASK_GPSIMD_EVERY", "2"))
# masked tiles: exp+mask fused on DVE (1) vs ACT exp + mask mult (0)
MASKED_ON_DVE = int(os.environ.get("K_MASKED_DVE", "0"))

_CACHE: dict = {}
_OPS: dict = {}


def _register_dve_ops():
    """Register the custom DVE exp ops (idempotent)."""
    if _OPS:
        return _OPS
    _q = Src0 * C0 + C1
    core_body = (sq(_q + C2) + One) * _q

    def _np_core(x, c0, c1, g):
        x = np.asarray(x, np.float32)
        qq = np.float32(c0) * x + np.float32(c1)
        return ((qq + np.float32(g)) ** 2 + np.float32(1.0)) * qq

    specs = [
        ("EXP_CORE_ANT", Spec(
            body=core_body,
            reference=lambda in0, in1, s0, s1, imm2: _np_core(in0, s0, s1, imm2),
        )),
        ("EXP_CORE_MASK_ANT", Spec(
            body=core_body * Src1,
            reference=lambda in0, in1, s0, s1, imm2: _np_core(in0, s0, s1, imm2)
            * np.asarray(in1, np.float32),
        )),
        ("EXP_SQ6_ANT", Spec(
            body=sq(sq(sq(sq(sq(sq(Src0)))))),
            reference=lambda in0, in1, s0, s1, imm2: (
                np.asarray(in0, np.float32) ** 64),
        )),
        # out = in0*s0 + in1*s1 with per-partition scalars: the fused
        # two-head output normalize+combine.
        ("OUT_COMBINE_ANT", Spec(
            body=Src0 * C0 + Src1 * C1,
            reference=lambda in0, in1, s0, s1, imm2: (
                np.asarray(in0, np.float32) * s0
                + np.asarray(in1, np.float32) * s1),
        )),
    ]
    for name, spec in specs:
        if name not in dve_ops._SUB_OPCODE_FOR_NAME:
            row = max(dve_ops._SUB_OPCODE_FOR_NAME.values()) + 1
            assert row < 0x20
            op = DveOp(name, spec, subdim=False, uops_sha={})
            for ver in ("v3", "v4"):
                s = DveOpSpec(name=name, opcode=row,
                              uops=lower(spec, ver=ver),
                              rd1_en=dspec._has_src1(spec))
                op.uops_sha[ver] = s.sha(ver)
            dve_ops.OPS.append(op)
            dve_ops.CUSTOM_DVE_SPECS[name] = spec
            dve_ops._SUB_OPCODE_FOR_NAME[name] = row
        _OPS[name] = next(o for o in dve_ops.OPS if o.name == name)
    return _OPS


def _classify_mask(mask: np.ndarray):
    """mask: [S, S] additive-style (nonzero => disallowed).

    Returns (schedule, patterns):
      schedule[qb] = list of (sk, qlo, pat_idx_or_None)
      patterns: np.ndarray [n_pat, 128, 512] of multiplicative 0/1 masks.
    """
    m = mask != 0
    schedule = []
    patterns = []
    pat_index: dict = {}
    for qb in range(N_QB):
        items = []
        for sk in range(N_SKT):
            sub = m[qb * SQ:(qb + 1) * SQ, sk * SK:(sk + 1) * SK].T
            if sub.all():
                continue
            if not sub.any():
                items.append((sk, 0, None))
                continue
            col_full_masked = sub.all(axis=0)
            qlo = int(np.argmax(~col_full_masked))
            qlo = (qlo // 128) * 128
            pat = (~sub).astype(np.float32)  # 1 = allowed
            key = pat.tobytes()
            if key not in pat_index:
                pat_index[key] = len(patterns)
                patterns.append(pat)
            items.append((sk, qlo, pat_index[key]))
        schedule.append(tuple(items))
    pats = np.stack(patterns) if patterns else np.ones((1, SK, SQ), np.float32)
    return tuple(schedule), pats


def _build_program(schedule, n_pat, reps=1):
    ops = _register_dve_ops()
    core_op = ops["EXP_CORE_ANT"]
    mask_op = ops["EXP_CORE_MASK_ANT"]
    sq6_op = ops["EXP_SQ6_ANT"]

    nc = bacc.Bacc("TRN2", target_bir_lowering=False, debug=False,
                   num_devices=N_CORES)

    qT = nc.dram_tensor("qT", [NF, S], BF16, kind="ExternalInput").ap()
    kT = nc.dram_tensor("kT", [NF, S], BF16, kind="ExternalInput").ap()
    vT = nc.dram_tensor("vT", [NF, S], BF16, kind="ExternalInput").ap()
    wq_d = nc.dram_tensor("wq", [NF, 128], BF16, kind="ExternalInput").ap()
    wk_d = nc.dram_tensor("wk", [NF, 128], BF16, kind="ExternalInput").ap()
    wv_d = nc.dram_tensor("wv", [NF, 128], BF16, kind="ExternalInput").ap()
    wo_d = nc.dram_tensor("wo", [128, NF], BF16, kind="ExternalInput").ap()
    bq_d = nc.dram_tensor("bq", [128, 1], F32, kind="ExternalInput").ap()
    bk_d = nc.dram_tensor("bk", [128, 1], F32, kind="ExternalInput").ap()
    msk_d = nc.dram_tensor("msk", [SK, n_pat * 2 * SQ], BF16,
                           kind="ExternalInput").ap()
    o_d = nc.dram_tensor("o", [S, NF], F32, kind="ExternalOutput").ap()

    with tile.TileContext(nc) as tc, ExitStack() as octx:
        per = octx.enter_context(tc.tile_pool(name="persist", bufs=1))

        QhT = per.tile([128, S], BF16, tag="qh")      # [head dims (A|B), S]
        KhT = per.tile([128, S], BF16, tag="kh")
        # PV stationaries, overlapping 128-wide windows per sk tile:
        #   cols 0:64 = A dims, col 64 = ones, 65:128 = 0, 128:192 = B dims
        #   A window = cols 0:128  (den -> psum row 64, attn rows 0:64)
        #   B window = cols 64:192 (den -> psum row 0, attn rows 64:128)
        # The single ones column serves both heads.
        Vaug = per.tile([128, N_SKT, 256], BF16, tag="vaug")
        # attnA: rows 0:64 attn, 64 = denA; attnB: row 0 = denB,
        # rows 64:128 attn (matches psum layout; lane-aligned copies).
        attnA = per.tile([128, S], BF16, tag="attnA")
        attnB = per.tile([128, S], BF16, tag="attnB")
        wq_sb = per.tile([128, 4, 128], BF16, tag="wq")
        wk_sb = per.tile([128, 4, 128], BF16, tag="wk")
        wv_sb = per.tile([128, 4, 128], BF16, tag="wv")
        wo_sb = per.tile([128, NF], BF16, tag="wo")
        bq_sb = per.tile([128, 1], F32, tag="bq")
        bk_sb = per.tile([128, 1], F32, tag="bk")
        msk_sb = per.tile([SK, n_pat, 2, SQ], BF16, tag="msk")
        ebias = per.tile([128, 1], F32, tag="ebias")
        ones_sb = per.tile([128, 1], BF16, tag="ones")

        nc.vector.memset(ebias, EXP_BIAS)
        nc.vector.memset(ones_sb, 1.0)
        nc.vector.memset(Vaug, 0.0)
        nc.vector.memset(Vaug[:, :, 64:65], 1.0)
        nc.sync.dma_start(wq_sb, wq_d.rearrange("(c p) m -> p c m", p=128))
        nc.sync.dma_start(wk_sb, wk_d.rearrange("(c p) m -> p c m", p=128))
        nc.sync.dma_start(wv_sb, wv_d.rearrange("(c p) m -> p c m", p=128))
        nc.sync.dma_start(wo_sb, wo_d)
        nc.sync.dma_start(bq_sb, bq_d)
        nc.sync.dma_start(bk_sb, bk_d)
        nc.sync.dma_start(
            msk_sb, msk_d.rearrange("k (p two q) -> k p two q", two=2, q=SQ))

        # PSUM banks: shared proj/oproj pool 2, lt 2x2=4, pv 2 -> 8.
        # Pools span the rep loop so the pipeline flows across reps.
        with tc.tile_pool(name="xs", bufs=3) as xs, \
             tc.tile_pool(name="ps2", bufs=2, space="PSUM") as ps2, \
             tc.tile_pool(name="pp", bufs=4) as pp, \
             tc.tile_pool(name="tp", bufs=2) as tp, \
             tc.tile_pool(name="ltp", bufs=2, space="PSUM") as ltp, \
             tc.tile_pool(name="pvp", bufs=2, space="PSUM") as pvp:
            dve_ctr = [0]
            msk_ctr = [0]

            def proj_dma(qb):
                """Issue the three input-block loads (early, so they
                prefetch ahead of the compute that consumes them)."""
                qsl = slice(qb * SQ, (qb + 1) * SQ)
                xk = xs.tile([128, 4, SQ], BF16, tag="x", bufs=4)
                nc.sync.dma_start(
                    xk, kT.rearrange("(c p) m -> p c m", p=128)[:, :, qsl])
                xq = xs.tile([128, 4, SQ], BF16, tag="x", bufs=4)
                nc.sync.dma_start(
                    xq, qT.rearrange("(c p) m -> p c m", p=128)[:, :, qsl])
                vb = xs.tile([128, 4, SQ], BF16, tag="x", bufs=4)
                nc.sync.dma_start(
                    vb, vT.rearrange("(c p) m -> p c m", p=128)[:, :, qsl])
                return xk, xq, vb

            def proj_mm(qb, tiles):
                qsl = slice(qb * SQ, (qb + 1) * SQ)
                xk, xq, vb = tiles
                for dst, xb, w_s, b_s in ((KhT, xk, wk_sb, bk_sb),
                                          (QhT, xq, wq_sb, bq_sb)):
                    pt = ps2.tile([128, SQ], F32, tag="ps")
                    for f in range(4):
                        nc.tensor.matmul(pt, w_s[:, f, :], xb[:, f, :],
                                         start=(f == 0), stop=(f == 3))
                    nc.vector.tensor_scalar_add(dst[:, qsl], pt, b_s)
                # V: x-stationary so psum comes out [s, d]; one strided
                # copy per 128-s chunk drops A dims into cols 0:64 and
                # B dims into cols 128:192.
                for j in range(4):
                    st = 4 * qb + j
                    pv_ = ps2.tile([128, 128], F32, tag="ps")
                    for f in range(4):
                        nc.tensor.matmul(pv_, vb[:, f, j * 128:(j + 1) * 128],
                                         wv_sb[:, f, :],
                                         start=(f == 0), stop=(f == 3))
                    nc.vector.tensor_copy(
                        Vaug[:, st, 0:256].rearrange(
                            "p (a b) -> p a b", a=2)[:, :, 0:64],
                        pv_.rearrange("p (a b) -> p a b", a=2))

            def emit_item(qb, idx, n_items, pvA, pvB):
                q0 = qb * SQ
                sk, qlo, pat = schedule[qb][idx]
                ksl = slice(sk * SK, (sk + 1) * SK)
                qs = slice(q0 + qlo, q0 + SQ)
                lt = ltp.tile([128, 1024], F32, tag="lt")
                pAB = pp.tile([128, 1024], BF16, tag="pAB")
                nc.tensor.matmul(lt[:, qlo:SQ], KhT[0:64, ksl],
                                 QhT[0:64, qs], start=True, stop=True)
                nc.tensor.matmul(lt[:, SQ + qlo:2 * SQ], KhT[64:128, ksl],
                                 QhT[64:128, qs], start=True, stop=True)
                if pat is None:
                    # full tile: route exp to ACT or DVE
                    dve_ctr[0] += 1
                    if dve_ctr[0] % DVE_FULL_EVERY == 0:
                        tmp = tp.tile([128, 1024], F32, tag="tmp")
                        nc.vector._custom_dve(
                            core_op, out=tmp, in0=lt,
                            s0=EC0, s1=EC1, imm2=EG)
                        nc.vector._custom_dve(sq6_op, out=pAB, in0=tmp)
                    else:
                        nc.scalar.activation(pAB, lt, AF.Exp,
                                             bias=ebias, scale=float(SCALE))
                elif MASKED_ON_DVE:
                    tmp = tp.tile([128, 1024], F32, tag="tmp")
                    t3 = tmp.rearrange("p (two q) -> p two q",
                                       q=SQ)[:, :, qlo:SQ]
                    oap = pAB.rearrange("p (two q) -> p two q",
                                        q=SQ)[:, :, qlo:SQ]
                    for h in range(2):
                        nc.vector._custom_dve(
                            mask_op, out=t3[:, h, :],
                            in0=lt[:, h * SQ + qlo:(h + 1) * SQ],
                            in1=msk_sb[:, pat, h, qlo:SQ],
                            s0=EC0, s1=EC1, imm2=EG)
                    nc.vector._custom_dve(sq6_op, out=oap, in0=t3)
                else:
                    oap = pAB.rearrange("p (two q) -> p two q",
                                        q=SQ)[:, :, qlo:SQ]
                    iap = lt.rearrange("p (two q) -> p two q",
                                       q=SQ)[:, :, qlo:SQ]
                    msl = msk_sb[:, pat, :, qlo:SQ]
                    nc.scalar.activation(oap, iap, AF.Exp,
                                         bias=ebias, scale=float(SCALE))
                    msk_ctr[0] += 1
                    if msk_ctr[0] % MASK_ON_GPSIMD_EVERY == 0:
                        nc.gpsimd.tensor_mul(oap, oap, msl)
                    else:
                        nc.vector.tensor_mul(oap, oap, msl)
                st_flag = (idx == 0)
                sp_flag = (idx == n_items - 1)
                nc.tensor.matmul(pvA[:, qlo:SQ], Vaug[:, sk, 0:128],
                                 pAB[:, qlo:SQ],
                                 start=st_flag, stop=sp_flag)
                nc.tensor.matmul(pvB[:, qlo:SQ], Vaug[:, sk, 64:192],
                                 pAB[:, SQ + qlo:2 * SQ],
                                 start=st_flag, stop=sp_flag)

            def emit_post(qb, pvA, pvB):
                qsl = slice(qb * SQ, (qb + 1) * SQ)
                if not schedule[qb]:
                    return
                nc.vector.tensor_copy(attnA[:, qsl], pvA[:, 0:SQ])
                nc.vector.tensor_copy(attnB[:, qsl], pvB[:, 0:SQ])

            def emit_oproj_st(qb, j):
                    st = 4 * qb + j
                    sl = slice(st * 128, (st + 1) * 128)
                    oA = ps2.tile([128, NF], F32, tag="ps")
                    oB = ps2.tile([128, NF], F32, tag="ps")
                    # denominators: 1-contraction matmuls pull the den
                    # rows (attnA row 64 / attnB row 0) into
                    # per-partition layout, borrowing col 0 of the
                    # oA/oB banks before the projection clobbers them
                    # (the recip read -> matmul WAR dep serializes).
                    rA = xs.tile([128, 1], F32, tag="r", bufs=4)
                    rB = xs.tile([128, 1], F32, tag="r", bufs=4)
                    nc.tensor.matmul(oA[:, 0:1], attnA[64:65, sl],
                                     ones_sb[64:65, :],
                                     start=True, stop=True)
                    nc.vector.reciprocal(rA, oA[:, 0:1])
                    nc.tensor.matmul(oB[:, 0:1], attnB[0:1, sl],
                                     ones_sb[0:1, :],
                                     start=True, stop=True)
                    nc.vector.reciprocal(rB, oB[:, 0:1])
                    nc.tensor.matmul(oA, attnA[0:64, sl], wo_sb[0:64, :],
                                     start=True, stop=True)
                    nc.tensor.matmul(oB, attnB[64:128, sl],
                                     wo_sb[64:128, :],
                                     start=True, stop=True)
                    t1 = xs.tile([128, NF], F32, tag="t1", bufs=2)
                    nc.vector.tensor_scalar_mul(t1, oB, rB)
                    osb = xs.tile([128, NF], F32, tag="os", bufs=2)
                    nc.vector.scalar_tensor_tensor(
                        osb, in0=oA, scalar=rA, in1=t1,
                        op0=ALU.mult, op1=ALU.add)
                    nc.sync.dma_start(o_d[sl, :], osb)

            # Per-rep emission, v6 ordering (best measured on HW): project
            # the next block ahead of the current block's attention, run
            # the output projection one block behind as a tail burst (its
            # den->recip->matmul chain then overlaps the next block's
            # independent attention work instead of head-of-line-blocking
            # the PE FIFO mid-stream).
            for _rep in range(reps):
                t0 = proj_dma(0)
                proj_mm(0, t0)
                for qb in range(N_QB):
                    pvA = pvp.tile([128, SQ], F32, tag="pv")
                    pvB = pvp.tile([128, SQ], F32, tag="pv")
                    if qb + 1 < N_QB:
                        t = proj_dma(qb + 1)
                        proj_mm(qb + 1, t)
                    n_items = len(schedule[qb])
                    for idx in range(n_items):
                        emit_item(qb, idx, n_items, pvA, pvB)
                    emit_post(qb, pvA, pvB)
                    if qb >= 1:
                        for j in range(4):
                            emit_oproj_st(qb - 1, j)
                for j in range(4):
                    emit_oproj_st(N_QB - 1, j)

    nc.compile()
    return nc


def _prep_core_inputs(c, q, k, v, wq, bq, wk, bk, wv, patterns):
    b = c // 4
    hp = c % 4
    cols = slice(128 * hp, 128 * (hp + 1))
    n_pat = patterns.shape[0]
    bf = ml_dtypes.bfloat16
    wo_slice = _prep_core_inputs._wo[cols, :]  # [128, 512]
    # patterns [n_pat, SK, SQ] -> [SK, n_pat, 2, SQ] (duplicated per head)
    mskd = np.repeat(patterns.transpose(1, 0, 2)[:, :, None, :], 2, axis=2)
    return {
        "qT": np.ascontiguousarray(q[b].T).astype(bf),
        "kT": np.ascontiguousarray(k[b].T).astype(bf),
        "vT": np.ascontiguousarray(v[b].T).astype(bf),
        "wq": np.ascontiguousarray(wq[:, cols]).astype(bf),
        "wk": np.ascontiguousarray(wk[:, cols]).astype(bf),
        "wv": np.ascontiguousarray(wv[:, cols]).astype(bf),
        "wo": np.ascontiguousarray(wo_slice).astype(bf),
        "bq": np.ascontiguousarray(bq[cols].reshape(128, 1)),
        "bk": np.ascontiguousarray(bk[cols].reshape(128, 1)),
        "msk": np.ascontiguousarray(
            mskd.reshape(SK, n_pat * 2 * SQ)).astype(bf),
    }


def get_state(mask_np, reps=1):
    """Build (or fetch cached) compiled program + schedule for this mask."""
    mask2d = np.asarray(mask_np, dtype=np.float32).reshape(S, S)
    schedule, patterns = _classify_mask(mask2d)
    key = (schedule, patterns.tobytes(), reps)
    if key not in _CACHE:
        nc = _build_program(schedule, patterns.shape[0], reps=reps)
        _CACHE[key] = {"nc": nc, "schedule": schedule, "patterns": patterns}
    return _CACHE[key]


def kernel(q, k, v, mask, wq, bq, wk, bk, wv, bv, wo, bo):
    q = np.asarray(q, np.float32)
    k = np.asarray(k, np.float32)
    v = np.asarray(v, np.float32)
    wq_n = np.asarray(wq, np.float32)
    wk_n = np.asarray(wk, np.float32)
    wv_n = np.asarray(wv, np.float32)
    wo_n = np.asarray(wo, np.float32)
    bq_n = np.asarray(bq, np.float32)
    bk_n = np.asarray(bk, np.float32)
    bv_n = np.asarray(bv, np.float32)
    bo_n = np.asarray(bo, np.float32)

    state = get_state(mask)
    nc = state["nc"]
    patterns = state["patterns"]

    _prep_core_inputs._wo = wo_n
    in_maps = [
        _prep_core_inputs(c, q, k, v, wq_n, bq_n, wk_n, bk_n, wv_n, patterns)
        for c in range(N_CORES)
    ]
    results = bass2jax.run_bass_via_pjrt(nc, in_maps, n_cores=N_CORES)

    bo_eff = bv_n @ wo_n + bo_n  # exact: softmax rows sum to 1
    out = np.empty((B, S, NF), np.float32)
    for b in range(B):
        acc = results[b * 4 + 0]["o"].astype(np.float32)
        for hp in range(1, 4):
            acc = acc + results[b * 4 + hp]["o"]
        out[b] = acc + bo_eff
    return out
